# revision 28
# baseline (speedup 1.0000x reference)
"""Multi-head causal attention (B=4, T=2048, D=1024, H=16) on 8 trn2 NeuronCores.

Sharding: core c handles batch b = c//2 and head-group g = c%2 (8 heads each).
Each core computes Q/K/V projections for its 8 heads, causal attention, and a
row-shard of the output projection; the host sums the two partial outputs per
batch (the "all-reduce") and adds the (b_v @ w_o + b_o) bias term.

Device algebra notes:
  - b_k drops out of softmax entirely (adds a per-query constant to scores).
  - b_v contributes exactly (b_v @ w_o) to every output row -> folded into the
    host-side bias along with b_o.
  - Scores are computed transposed ([k, q] layout) so that softmax sums ride
    free on the AV matmul (ones-column appended to V) and the AV output comes
    out as AV^T, which feeds the w_o matmul with no extra transposes.

DMA rule: hardware DMA descriptors encode at most ONE semaphore wait, so every
DMA destination here is written exactly once (no pool-slot reuse for DMA
targets); partition broadcasts are done with PE outer products, not DMA.
"""

import math
from contextlib import ExitStack

import numpy as np

import concourse.bass as bass
import concourse.mybir as mybir
import concourse.tile as tile
from concourse import bacc
from concourse.bass_utils import run_bass_kernel_spmd


FP = mybir.dt.float32
BF = mybir.dt.bfloat16

D_MODEL = 1024
N_HEADS = 16
B_FULL, T_FULL = 4, 2048
DK = 64                    # head dim
HPC = 8                    # heads per core
DH = HPC * DK              # 512 head-dims per core
N_CORES = 8


def build_bass(seq_len=T_FULL, causal=True, repeat=1, stages='123F'):
    """Build the per-core Bass kernel (SPMD; same NEFF on all 8 cores).

    repeat > 1 wraps the whole kernel in a hardware loop — used only for
    benchmarking (amortizes host dispatch to time the kernel itself).
    """
    T = seq_len
    NT = T // 128             # t-tiles
    NCH = T // 512            # 512-wide t/q chunks
    ND = D_MODEL // 128       # d_model tiles (8)
    NM = DH // 128            # head-pair tiles (4)

    nc = bacc.Bacc("TRN2", target_bir_lowering=False, debug=False)
    # x arrives pre-transposed AND pre-cast to bf16 (host does both), so the
    # xT tile loads are plain partition-contiguous DMAs, not transpose DMAs.
    xt_d = nc.dram_tensor("xt", [D_MODEL, T], BF, kind="ExternalInput")
    wq_d = nc.dram_tensor("wq", [D_MODEL, DH], BF, kind="ExternalInput")
    wk_d = nc.dram_tensor("wk", [D_MODEL, DH], BF, kind="ExternalInput")
    wv_d = nc.dram_tensor("wv", [D_MODEL, DH], BF, kind="ExternalInput")
    wo_d = nc.dram_tensor("wo", [DH, D_MODEL], BF, kind="ExternalInput")
    bq_d = nc.dram_tensor("bq", [128, DH // 128], FP, kind="ExternalInput")
    out_d = nc.dram_tensor("out", [T, D_MODEL], BF, kind="ExternalOutput")

    with ExitStack() as ctx:
        tc = ctx.enter_context(tile.TileContext(nc))
        persist = ctx.enter_context(tc.tile_pool(name="persist", bufs=1))
        qt_pool = ctx.enter_context(tc.tile_pool(name="qt", bufs=3))
        xt_pool = ctx.enter_context(tc.tile_pool(name="xt", bufs=3))
        at_pool = ctx.enter_context(tc.tile_pool(name="atp", bufs=8))
        rec_pool = ctx.enter_context(tc.tile_pool(name="rec", bufs=2))
        avn_pool = ctx.enter_context(tc.tile_pool(name="avn", bufs=2))
        out_pool = ctx.enter_context(tc.tile_pool(name="outp", bufs=3))
        mm_ps = ctx.enter_context(tc.tile_pool(name="mmps", bufs=2, space="PSUM"))
        sc_ps = ctx.enter_context(tc.tile_pool(name="scps", bufs=2, space="PSUM"))
        av_ps = ctx.enter_context(tc.tile_pool(name="avps", bufs=2, space="PSUM"))

        def emit_kernel():
            # ---- constants -------------------------------------------------
            ones_bf = persist.tile([1, DK], BF, name="ones_bf", tag="ones_bf")
            nc.gpsimd.memset(ones_bf, 1.0)

            mask_tri = None
            if causal:
                # one shared lower-triangular keep mask for the diagonal
                # 128x128 block: keep where q_local >= k
                mask_tri = persist.tile([128, 128], BF, name="mask_tri",
                                        tag="mask_tri")
                nc.gpsimd.memset(mask_tri, 1.0)
                nc.gpsimd.affine_select(
                    out=mask_tri, in_=mask_tri,
                    compare_op=mybir.AluOpType.is_ge,
                    fill=0.0, base=0, pattern=[[1, 128]],
                    channel_multiplier=-1)

            def bcast_mid(ap, nmid):
                return bass.AP(tensor=ap.tensor, offset=ap.offset,
                               ap=[list(ap.ap[0]), [0, nmid], list(ap.ap[1])])

            def emit_s1(n):
                xT = [xt_pool.tile([128, 512], BF, name=f"xT{j}",
                                   tag=f"xT{j}") for j in range(ND)]
                for j in range(ND):
                    nc.sync.dma_start(out=xT[j],
                                      in_=xt_d[j * 128:(j + 1) * 128,
                                               n * 512:(n + 1) * 512])
                return xT

            # queue chunk-0 x tiles ahead of the weights: the first kgroup
            # needs xT(0) and wk, and the 8 HWDGE queues drain round-robin
            xT_first = emit_s1(0)

            # ---- weights: one DMA per 128-row tile, spread across the 8
            # HWDGE queues (per-queue bandwidth is ~1/16 of aggregate, so a
            # single 1MB DMA would serialize ~26us at startup)
            def load_weight_bf(dram, n_tiles, cols, label):
                wsb = persist.tile([128, n_tiles, cols], BF, name=f"w_{label}",
                                   tag=f"w_{label}")
                dap = dram.ap().rearrange("(j p) c -> p j c", p=128)
                for j in range(n_tiles):
                    nc.sync.dma_start(out=wsb[:, j, :], in_=dap[:, j, :])
                return [wsb[:, j, :] for j in range(n_tiles)]

            wk_bf = load_weight_bf(wk_d, ND, DH, "k")
            wq_bf = load_weight_bf(wq_d, ND, DH, "q")
            wv_bf = load_weight_bf(wv_d, ND, DH, "v")
            wo_bf = load_weight_bf(wo_d, NM, D_MODEL, "o")

            bq_sb = persist.tile([128, NM], FP, name="bq_sb", tag="bq_sb")
            nc.sync.dma_start(out=bq_sb, in_=bq_d[:, :])

            # ---- persistent per-core tensors ------------------------------
            # KT[mt]: [128, T] bf16, rows = head-dim (pair mt: heads 2mt,2mt+1)
            KT = [persist.tile([128, T], BF, name=f"KT{mt}", tag=f"KT{mt}")
                  for mt in range(NM)]
            # V natural layout with ones column: [128 t, 8 heads, 64+1]
            V_sb = [persist.tile([128, HPC, DK + 1], BF, name=f"V{i}", tag=f"V{i}")
                    for i in range(NT)]
            for i in range(NT):
                nc.gpsimd.memset(V_sb[i][:, :, DK], 1.0)
            # AV^T, write-once (DMA target for the h1 partition shift)
            AVT_all = [[persist.tile([128, 512], BF, name=f"AVT{n}_{p}",
                                     tag=f"AVT{n}_{p}") for p in range(NM)]
                       for n in range(NCH)]

            # ---- main streamed loop over 512-wide chunks -------------------
            # S2(n+1) and F(n-1) matmul groups are fed into S3(n)'s pair
            # boundaries: the per-engine queues are in-order, so S3's
            # exp-latency bubbles can only be filled by positionally
            # interleaving independent work into its instruction stream.

            def make_s2_groups(n, xT):
                csl = bass.ts(n, 512)
                QT = [qt_pool.tile([128, 512], BF, name=f"QT{mt}",
                                   tag=f"QT{mt}") for mt in range(NM)]
                groups = []

                def qgroup(mt):
                    msl = bass.ts(mt, 128)
                    ps = mm_ps.tile([128, 512], FP, name="psq", tag="mm")
                    for j in range(ND):
                        nc.tensor.matmul(ps, wq_bf[j][:, msl], xT[j],
                                         start=(j == 0), stop=(j == ND - 1))
                    nc.vector.tensor_scalar_add(QT[mt], ps, bq_sb[:, mt:mt + 1])

                def kgroup(mt):
                    msl = bass.ts(mt, 128)
                    ps = mm_ps.tile([128, 512], FP, name="psk", tag="mm")
                    for j in range(ND):
                        nc.tensor.matmul(ps, wk_bf[j][:, msl], xT[j],
                                         start=(j == 0), stop=(j == ND - 1))
                    # fold the 1/sqrt(dk) score scale into K^T so exp needs
                    # no scale and score magnitudes stay ~N(0,1)
                    nc.vector.tensor_scalar_mul(KT[mt][:, csl], ps,
                                                1.0 / math.sqrt(DK))

                def vgroup(il):
                    i = 4 * n + il
                    ps = mm_ps.tile([128, 512], FP, name="psv", tag="mm")
                    for j in range(ND):
                        nc.tensor.matmul(ps, xT[j][:, il * 128:(il + 1) * 128],
                                         wv_bf[j], start=(j == 0),
                                         stop=(j == ND - 1))
                    nc.vector.tensor_copy(
                        V_sb[i][:, :, 0:DK],
                        ps.rearrange("p (h d) -> p h d", h=HPC))

                if '2' in stages:
                    for mt in range(NM):
                        groups.append(lambda mt=mt: kgroup(mt))
                        groups.append(lambda mt=mt: qgroup(mt))
                    for il in range(4):
                        groups.append(lambda il=il: vgroup(il))
                return QT, groups

            def make_f_groups(n):
                AVT = AVT_all[n]
                groups = []

                osbs = {}

                def fgroup(il, cc):
                    i = 4 * n + il
                    isl = bass.ts(il, 128)
                    if il not in osbs:
                        osbs[il] = out_pool.tile([128, D_MODEL], BF,
                                                 name="osb", tag="osb")
                    osb = osbs[il]
                    ps = mm_ps.tile([128, 512], FP, name="pso", tag="mm")
                    for dk in range(NM):
                        nc.tensor.matmul(
                            ps, AVT[dk][:, isl],
                            wo_bf[dk][:, cc * 512:(cc + 1) * 512],
                            start=(dk == 0), stop=(dk == NM - 1))
                    csl2 = slice(cc * 512, (cc + 1) * 512)
                    nc.vector.tensor_copy(osb[:, csl2], ps)
                    # half-width bf16 DMA per cc: keeps per-queue occupancy
                    # short so xT prefetches behind it aren't head-blocked
                    nc.sync.dma_start(
                        out=out_d[i * 128:(i + 1) * 128, csl2],
                        in_=osb[:, csl2])

                if 'F' in stages:
                    for il in range(4):
                        for cc in range(2):
                            groups.append(lambda il=il, cc=cc: fgroup(il, cc))
                return groups

            def emit_s3(n, QT, feed):
                AVT = AVT_all[n]
                nkt = 4 * n + 4 if causal else NT
                PIPE = 4
                pending_norm = [None]

                def emit_norm(p, av0, av1):
                    den_bf = rec_pool.tile([1, 2, 512], BF, name="den_bf",
                                           tag="den_bf")
                    nc.vector.tensor_copy(den_bf[:, 0, :], av0[DK:DK + 1, :])
                    nc.vector.tensor_copy(den_bf[:, 1, :], av1[DK:DK + 1, :])
                    rb_sb = rec_pool.tile([DK, 2, 512], FP, name="rb_sb",
                                          tag="rb_sb")
                    for hh in range(2):
                        rb = mm_ps.tile([DK, 512], FP, name=f"rb{hh}",
                                        tag="mm")
                        nc.tensor.matmul(rb, ones_bf, den_bf[:, hh, :],
                                         start=True, stop=True)
                        nc.vector.reciprocal(rb_sb[:, hh, :], rb)
                    nc.vector.tensor_mul(AVT[p][0:64, :], av0[0:DK, :],
                                         rb_sb[:, 0, :])
                    avn1 = avn_pool.tile([64, 512], BF, name="avn1",
                                         tag="avn1")
                    nc.vector.tensor_mul(avn1, av1[0:DK, :], rb_sb[:, 1, :])
                    nc.sync.dma_start(out=AVT[p][64:128, :], in_=avn1)

                npairs = NM if '3' in stages else 0
                for p in range(npairs):
                    av0 = av_ps.tile([DK + 1, 512], FP, name="av0", tag="av")
                    av1 = av_ps.tile([DK + 1, 512], FP, name="av1", tag="av")
                    avs = (av0, av1)

                    def emit_av(idx, kt, at, q0):
                        for hh in range(2):
                            nc.tensor.matmul(
                                avs[hh][:, q0:512],
                                V_sb[kt][:, 2 * p + hh, :],
                                at[:, hh, q0:512],
                                start=(idx == 0), stop=(idx == nkt - 1),
                                skip_group_check=True)

                    # process diagonal (masked, partial-width) tiles FIRST so
                    # their longer exp+mask chains hide under the full tiles;
                    # diag j=0 is full-width so start= covers all columns
                    if causal:
                        kt_order = list(range(4 * n, nkt)) + list(range(4 * n))
                    else:
                        kt_order = list(range(nkt))

                    pend = []
                    for idx, kt in enumerate(kt_order):
                        ksl = bass.ts(kt, 128)
                        # columns < j*128 of a diagonal tile are fully
                        # masked: skip in scores matmul / exp / mask / AV
                        j = kt - 4 * n if (causal and kt >= 4 * n) else 0
                        q0 = j * 128
                        ps_s = sc_ps.tile([128, 2, 512], FP, name="ps_s",
                                          tag="sc")
                        at = at_pool.tile([128, 2, 512], BF, name="at",
                                          tag="at")
                        for hh in range(2):
                            nc.tensor.matmul(
                                ps_s[:, hh, q0:512],
                                KT[p][hh * 64:(hh + 1) * 64, ksl],
                                QT[p][hh * 64:(hh + 1) * 64, q0:512],
                                start=True, stop=True,
                                tile_position=(hh * 64, 0))
                        nc.scalar.activation(at[:, :, q0:512],
                                             ps_s[:, :, q0:512],
                                             mybir.ActivationFunctionType.Exp)
                        if causal and kt >= 4 * n:
                            # only the diagonal 128-col block needs masking;
                            # columns right of it are fully unmasked
                            nc.vector.tensor_mul(
                                at[:, :, q0:q0 + 128], at[:, :, q0:q0 + 128],
                                bcast_mid(mask_tri[:, :], 2))
                        if idx == 0 and pending_norm[0] is not None:
                            pending_norm[0]()
                            pending_norm[0] = None
                        pend.append((idx, kt, at, q0))
                        if len(pend) > PIPE:
                            emit_av(*pend.pop(0))
                        # drip independent S2/F work into the exp-paced
                        # attention chain: PE has ~0.3us idle per kt step
                        if feed and idx % 6 == 3:
                            feed.pop(0)()
                    for item in pend:
                        emit_av(*item)
                    pending_norm[0] = (lambda p=p, a0=av0, a1=av1:
                                       emit_norm(p, a0, a1))
                    # fill the pair-transition bubble with independent work
                    for _ in range(5):
                        if feed:
                            feed.pop(0)()
                if pending_norm[0] is not None:
                    pending_norm[0]()
                    pending_norm[0] = None
                while feed:
                    feed.pop(0)()

            xT_cur = xT_first
            QT_cur, s2g = make_s2_groups(0, xT_cur)
            for g in s2g:
                g()
            f_prev = []
            for n in range(NCH):
                feed = []
                if n + 1 < NCH:
                    xT_next = emit_s1(n + 1)
                    QT_next, s2g_next = make_s2_groups(n + 1, xT_next)
                    feed += s2g_next
                else:
                    QT_next = None
                feed += f_prev
                if '3' in stages:
                    emit_s3(n, QT_cur, feed)
                else:
                    for g in feed:
                        g()
                f_prev = make_f_groups(n)
                QT_cur = QT_next
            for g in f_prev:
                g()

        if repeat > 1:
            with tc.For_i(0, repeat, 1):
                emit_kernel()
        else:
            emit_kernel()

    nc.compile()
    return nc


_NC_CACHE = {}


def _get_nc(seq_len, causal):
    key = (seq_len, causal)
    if key not in _NC_CACHE:
        _NC_CACHE[key] = build_bass(seq_len, causal)
    return _NC_CACHE[key]


def make_in_maps(x, w_q, b_q, w_k, w_v, w_o):
    """Per-core input dicts for the 8 cores (weights/x pre-cast to bf16)."""
    import ml_dtypes
    bf = ml_dtypes.bfloat16
    xt_bf = np.ascontiguousarray(x.transpose(0, 2, 1)).astype(bf)
    wq_bf = w_q.astype(bf)
    wk_bf = w_k.astype(bf)
    wv_bf = w_v.astype(bf)
    wo_bf = w_o.astype(bf)
    in_maps = []
    for c in range(N_CORES):
        b, g = divmod(c, 2)
        sl = slice(g * DH, (g + 1) * DH)
        in_maps.append({
            "xt": xt_bf[b],
            "wq": np.ascontiguousarray(wq_bf[:, sl]),
            "wk": np.ascontiguousarray(wk_bf[:, sl]),
            "wv": np.ascontiguousarray(wv_bf[:, sl]),
            "wo": np.ascontiguousarray(wo_bf[sl, :]),
            "bq": np.ascontiguousarray(
                b_q[sl].reshape(DH // 128, 128).T.astype(np.float32)),
        })
    return in_maps


def kernel(x, mask, w_q, b_q, w_k, b_k, w_v, b_v, w_o, b_o, _trace=False):
    x = np.asarray(x, dtype=np.float32)
    mask_np = np.asarray(mask).reshape(mask.shape[-2], mask.shape[-1])
    w_q, b_q = np.asarray(w_q, np.float32), np.asarray(b_q, np.float32)
    w_k = np.asarray(w_k, np.float32)
    w_v, b_v = np.asarray(w_v, np.float32), np.asarray(b_v, np.float32)
    w_o, b_o = np.asarray(w_o, np.float32), np.asarray(b_o, np.float32)

    T = x.shape[1]
    tril = np.tril(np.ones((T, T), dtype=mask_np.dtype))
    if np.array_equal(mask_np, tril):
        causal = True
    elif np.all(mask_np != 0):
        causal = False
    else:
        raise NotImplementedError("only causal or all-ones masks supported")

    nc = _get_nc(T, causal)
    in_maps = make_in_maps(x, w_q, b_q, w_k, w_v, w_o)
    res = run_bass_kernel_spmd(nc, in_maps, core_ids=list(range(N_CORES)),
                               trace=_trace)

    host_bias = (b_v @ w_o + b_o).astype(np.float32)
    out = np.empty((x.shape[0], T, D_MODEL), dtype=np.float32)
    for b in range(x.shape[0]):
        out[b] = res.results[2 * b]["out"].astype(np.float32) \
            + res.results[2 * b + 1]["out"].astype(np.float32) + host_bias
    kernel._last_result = res
    return out



# revision 35
# speedup vs baseline: 1.1072x; 1.1072x over previous
"""Multi-head causal attention (B=4, T=2048, D=1024, H=16) on 8 trn2 NeuronCores.

Sharding: core c handles batch b = c//2 and head-group g = c%2 (8 heads each).
Each core computes Q/K/V projections for its 8 heads, causal attention, and a
row-shard of the output projection; the host sums the two partial outputs per
batch (the "all-reduce") and adds the (b_v @ w_o + b_o) bias term.

Device algebra notes:
  - b_k drops out of softmax entirely (adds a per-query constant to scores).
  - b_v contributes exactly (b_v @ w_o) to every output row -> folded into the
    host-side bias along with b_o.
  - Scores are computed transposed ([k, q] layout) so that softmax sums ride
    free on the AV matmul (ones-column appended to V) and the AV output comes
    out as AV^T, which feeds the w_o matmul with no extra transposes.

DMA rule: hardware DMA descriptors encode at most ONE semaphore wait, so every
DMA destination here is written exactly once (no pool-slot reuse for DMA
targets); partition broadcasts are done with PE outer products, not DMA.
"""

import math
from contextlib import ExitStack

import numpy as np

import concourse.bass as bass
import concourse.mybir as mybir
import concourse.tile as tile
from concourse import bacc
from concourse.bass_utils import run_bass_kernel_spmd


FP = mybir.dt.float32
BF = mybir.dt.bfloat16

D_MODEL = 1024
N_HEADS = 16
B_FULL, T_FULL = 4, 2048
DK = 64                    # head dim
HPC = 8                    # heads per core
DH = HPC * DK              # 512 head-dims per core
N_CORES = 8


def build_bass(seq_len=T_FULL, causal=True, repeat=1, stages='123F'):
    """Build the per-core Bass kernel (SPMD; same NEFF on all 8 cores).

    repeat > 1 wraps the whole kernel in a hardware loop — used only for
    benchmarking (amortizes host dispatch to time the kernel itself).
    """
    T = seq_len
    NT = T // 128             # t-tiles
    NCH = T // 512            # 512-wide t/q chunks
    ND = D_MODEL // 128       # d_model tiles (8)
    NM = DH // 128            # head-pair tiles (4)

    nc = bacc.Bacc("TRN2", target_bir_lowering=False, debug=False)
    # x arrives pre-transposed AND pre-cast to bf16 (host does both), so the
    # xT tile loads are plain partition-contiguous DMAs, not transpose DMAs.
    xt_d = nc.dram_tensor("xt", [D_MODEL, T], BF, kind="ExternalInput")
    wq_d = nc.dram_tensor("wq", [D_MODEL, DH], BF, kind="ExternalInput")
    wk_d = nc.dram_tensor("wk", [D_MODEL, DH], BF, kind="ExternalInput")
    wv_d = nc.dram_tensor("wv", [D_MODEL, DH], BF, kind="ExternalInput")
    wo_d = nc.dram_tensor("wo", [DH, D_MODEL], BF, kind="ExternalInput")
    bq_d = nc.dram_tensor("bq", [128, DH // 128], FP, kind="ExternalInput")
    out_d = nc.dram_tensor("out", [T, D_MODEL], BF, kind="ExternalOutput")

    with ExitStack() as ctx:
        tc = ctx.enter_context(tile.TileContext(nc))
        persist = ctx.enter_context(tc.tile_pool(name="persist", bufs=1))
        qt_pool = ctx.enter_context(tc.tile_pool(name="qt", bufs=3))
        xt_pool = ctx.enter_context(tc.tile_pool(name="xt", bufs=3))
        at_pool = ctx.enter_context(tc.tile_pool(name="atp", bufs=6))
        rec_pool = ctx.enter_context(tc.tile_pool(name="rec", bufs=2))
        avn_pool = ctx.enter_context(tc.tile_pool(name="avn", bufs=2))
        out_pool = ctx.enter_context(tc.tile_pool(name="outp", bufs=3))
        mm_ps = ctx.enter_context(tc.tile_pool(name="mmps", bufs=2, space="PSUM"))
        sc_ps = ctx.enter_context(tc.tile_pool(name="scps", bufs=2, space="PSUM"))
        av_ps = ctx.enter_context(tc.tile_pool(name="avps", bufs=2, space="PSUM"))

        def emit_kernel():
            # ---- constants -------------------------------------------------
            ones_bf = persist.tile([1, DK], BF, name="ones_bf", tag="ones_bf")
            nc.gpsimd.memset(ones_bf, 1.0)

            mask_tri = None
            if causal:
                # one shared lower-triangular keep mask for the diagonal
                # 128x128 block: keep where q_local >= k
                mask_tri = persist.tile([128, 128], BF, name="mask_tri",
                                        tag="mask_tri")
                nc.gpsimd.memset(mask_tri, 1.0)
                nc.gpsimd.affine_select(
                    out=mask_tri, in_=mask_tri,
                    compare_op=mybir.AluOpType.is_ge,
                    fill=0.0, base=0, pattern=[[1, 128]],
                    channel_multiplier=-1)

            def bcast_mid(ap, nmid):
                return bass.AP(tensor=ap.tensor, offset=ap.offset,
                               ap=[list(ap.ap[0]), [0, nmid], list(ap.ap[1])])

            def emit_s1(n):
                xT = [xt_pool.tile([128, 512], BF, name=f"xT{j}",
                                   tag=f"xT{j}") for j in range(ND)]
                for j in range(ND):
                    nc.sync.dma_start(out=xT[j],
                                      in_=xt_d[j * 128:(j + 1) * 128,
                                               n * 512:(n + 1) * 512])
                return xT

            # queue chunk-0 x tiles ahead of the weights: the first kgroup
            # needs xT(0) and wk, and the 8 HWDGE queues drain round-robin
            xT_first = emit_s1(0)

            # ---- weights: one DMA per 128-row tile, spread across the 8
            # HWDGE queues (per-queue bandwidth is ~1/16 of aggregate, so a
            # single 1MB DMA would serialize ~26us at startup)
            def load_weight_bf(dram, n_tiles, cols, label):
                wsb = persist.tile([128, n_tiles, cols], BF, name=f"w_{label}",
                                   tag=f"w_{label}")
                dap = dram.ap().rearrange("(j p) c -> p j c", p=128)
                for j in range(n_tiles):
                    nc.sync.dma_start(out=wsb[:, j, :], in_=dap[:, j, :])
                return [wsb[:, j, :] for j in range(n_tiles)]

            wk_bf = load_weight_bf(wk_d, ND, DH, "k")
            wq_bf = load_weight_bf(wq_d, ND, DH, "q")
            wv_bf = load_weight_bf(wv_d, ND, DH, "v")
            wo_bf = load_weight_bf(wo_d, NM, D_MODEL, "o")

            bq_sb = persist.tile([128, NM], FP, name="bq_sb", tag="bq_sb")
            nc.sync.dma_start(out=bq_sb, in_=bq_d[:, :])

            # ---- persistent per-core tensors ------------------------------
            # KT[mt]: [128, T] bf16, rows = head-dim (pair mt: heads 2mt,2mt+1)
            KT = [persist.tile([128, T], BF, name=f"KT{mt}", tag=f"KT{mt}")
                  for mt in range(NM)]
            # V natural layout with ones column: [128 t, 8 heads, 64+1]
            V_sb = [persist.tile([128, HPC, DK + 1], BF, name=f"V{i}", tag=f"V{i}")
                    for i in range(NT)]
            for i in range(NT):
                nc.gpsimd.memset(V_sb[i][:, :, DK], 1.0)
            # AV^T, write-once (DMA target for the h1 partition shift)
            AVT_all = [[persist.tile([128, 512], BF, name=f"AVT{n}_{p}",
                                     tag=f"AVT{n}_{p}") for p in range(NM)]
                       for n in range(NCH)]

            # ---- main streamed loop over 512-wide chunks -------------------
            # S2(n+1) and F(n-1) matmul groups are fed into S3(n)'s pair
            # boundaries: the per-engine queues are in-order, so S3's
            # exp-latency bubbles can only be filled by positionally
            # interleaving independent work into its instruction stream.

            def make_s2_groups(n, xT):
                csl = bass.ts(n, 512)
                QT = [persist.tile([128, 512], BF, name=f"QT{n}_{mt}",
                                   tag=f"QT{n}_{mt}") for mt in range(NM)]
                groups = []

                def qgroup(mt):
                    msl = bass.ts(mt, 128)
                    ps = mm_ps.tile([128, 512], FP, name="psq", tag="mm")
                    for j in range(ND):
                        nc.tensor.matmul(ps, wq_bf[j][:, msl], xT[j],
                                         start=(j == 0), stop=(j == ND - 1))
                    nc.vector.tensor_scalar_add(QT[mt], ps, bq_sb[:, mt:mt + 1])

                def kgroup(mt):
                    msl = bass.ts(mt, 128)
                    ps = mm_ps.tile([128, 512], FP, name="psk", tag="mm")
                    for j in range(ND):
                        nc.tensor.matmul(ps, wk_bf[j][:, msl], xT[j],
                                         start=(j == 0), stop=(j == ND - 1))
                    # fold the 1/sqrt(dk) score scale into K^T so exp needs
                    # no scale and score magnitudes stay ~N(0,1)
                    nc.vector.tensor_scalar_mul(KT[mt][:, csl], ps,
                                                1.0 / math.sqrt(DK))

                def vgroup(il):
                    i = 4 * n + il
                    ps = mm_ps.tile([128, 512], FP, name="psv", tag="mm")
                    for j in range(ND):
                        nc.tensor.matmul(ps, xT[j][:, il * 128:(il + 1) * 128],
                                         wv_bf[j], start=(j == 0),
                                         stop=(j == ND - 1))
                    nc.vector.tensor_copy(
                        V_sb[i][:, :, 0:DK],
                        ps.rearrange("p (h d) -> p h d", h=HPC))

                if '2' in stages:
                    for mt in range(NM):
                        groups.append(lambda mt=mt: kgroup(mt))
                        groups.append(lambda mt=mt: qgroup(mt))
                    for il in range(4):
                        groups.append(lambda il=il: vgroup(il))
                return QT, groups

            def make_f_groups(n):
                AVT = AVT_all[n]
                groups = []

                osbs = {}

                def fgroup(il, cc):
                    i = 4 * n + il
                    isl = bass.ts(il, 128)
                    if il not in osbs:
                        osbs[il] = out_pool.tile([128, D_MODEL], BF,
                                                 name="osb", tag="osb")
                    osb = osbs[il]
                    ps = mm_ps.tile([128, 512], FP, name="pso", tag="mm")
                    for dk in range(NM):
                        nc.tensor.matmul(
                            ps, AVT[dk][:, isl],
                            wo_bf[dk][:, cc * 512:(cc + 1) * 512],
                            start=(dk == 0), stop=(dk == NM - 1))
                    csl2 = slice(cc * 512, (cc + 1) * 512)
                    nc.vector.tensor_copy(osb[:, csl2], ps)
                    # half-width bf16 DMA per cc: keeps per-queue occupancy
                    # short so xT prefetches behind it aren't head-blocked
                    nc.sync.dma_start(
                        out=out_d[i * 128:(i + 1) * 128, csl2],
                        in_=osb[:, csl2])

                if 'F' in stages:
                    for il in range(4):
                        for cc in range(2):
                            groups.append(lambda il=il, cc=cc: fgroup(il, cc))
                return groups

            def emit_s3(n, QT, feed):
                AVT = AVT_all[n]
                nkt = 4 * n + 4 if causal else NT
                PIPE = 3
                pending_norm = [None]

                def emit_norm(p, av0, av1):
                    den_bf = rec_pool.tile([1, 2, 512], BF, name="den_bf",
                                           tag="den_bf")
                    nc.vector.tensor_copy(den_bf[:, 0, :], av0[DK:DK + 1, :])
                    nc.vector.tensor_copy(den_bf[:, 1, :], av1[DK:DK + 1, :])
                    rb_sb = rec_pool.tile([DK, 2, 512], FP, name="rb_sb",
                                          tag="rb_sb")
                    for hh in range(2):
                        rb = mm_ps.tile([DK, 512], FP, name=f"rb{hh}",
                                        tag="mm")
                        nc.tensor.matmul(rb, ones_bf, den_bf[:, hh, :],
                                         start=True, stop=True)
                        nc.vector.reciprocal(rb_sb[:, hh, :], rb)
                    nc.vector.tensor_mul(AVT[p][0:64, :], av0[0:DK, :],
                                         rb_sb[:, 0, :])
                    avn1 = avn_pool.tile([64, 512], BF, name="avn1",
                                         tag="avn1")
                    nc.vector.tensor_mul(avn1, av1[0:DK, :], rb_sb[:, 1, :])
                    nc.sync.dma_start(out=AVT[p][64:128, :], in_=avn1)

                npairs = NM if '3' in stages else 0
                for p in range(npairs):
                    av0 = av_ps.tile([DK + 1, 512], FP, name="av0", tag="av")
                    av1 = av_ps.tile([DK + 1, 512], FP, name="av1", tag="av")
                    avs = (av0, av1)

                    def emit_av(idx, kt, at, q0):
                        for hh in range(2):
                            nc.tensor.matmul(
                                avs[hh][:, q0:512],
                                V_sb[kt][:, 2 * p + hh, :],
                                at[:, hh, q0:512],
                                start=(idx == 0), stop=(idx == nkt - 1),
                                skip_group_check=True)

                    # process diagonal (masked, partial-width) tiles FIRST so
                    # their longer exp+mask chains hide under the full tiles;
                    # diag j=0 is full-width so start= covers all columns
                    if causal:
                        kt_order = list(range(4 * n, nkt)) + list(range(4 * n))
                    else:
                        kt_order = list(range(nkt))

                    pend = []
                    for idx, kt in enumerate(kt_order):
                        ksl = bass.ts(kt, 128)
                        # pop the ready AV FIRST: the PE queue is in-order,
                        # so a scores matmul blocked on its psum bank must
                        # not sit ahead of AV work that can already run
                        if len(pend) > PIPE - 1:
                            emit_av(*pend.pop(0))
                        # columns < j*128 of a diagonal tile are fully
                        # masked: skip in scores matmul / exp / mask / AV
                        j = kt - 4 * n if (causal and kt >= 4 * n) else 0
                        q0 = j * 128
                        ps_s = sc_ps.tile([128, 2, 512], FP, name="ps_s",
                                          tag="sc")
                        at = at_pool.tile([128, 2, 512], BF, name="at",
                                          tag="at")
                        for hh in range(2):
                            nc.tensor.matmul(
                                ps_s[:, hh, q0:512],
                                KT[p][hh * 64:(hh + 1) * 64, ksl],
                                QT[p][hh * 64:(hh + 1) * 64, q0:512],
                                start=True, stop=True,
                                tile_position=(hh * 64, 0))
                        nc.scalar.activation(at[:, :, q0:512],
                                             ps_s[:, :, q0:512],
                                             mybir.ActivationFunctionType.Exp)
                        if causal and kt >= 4 * n:
                            # only the diagonal 128-col block needs masking;
                            # columns right of it are fully unmasked
                            nc.vector.tensor_mul(
                                at[:, :, q0:q0 + 128], at[:, :, q0:q0 + 128],
                                bcast_mid(mask_tri[:, :], 2))
                        # deferred norm of the previous pair: fire a couple of
                        # steps in so its PE broadcasts (waiting on DVE den
                        # copies) don't head-block this pair's early scores
                        if idx == min(2, nkt - 1) and \
                                pending_norm[0] is not None:
                            pending_norm[0]()
                            pending_norm[0] = None
                        pend.append((idx, kt, at, q0))
                    for item in pend:
                        emit_av(*item)
                    pending_norm[0] = (lambda p=p, a0=av0, a1=av1:
                                       emit_norm(p, a0, a1))
                    # fill the pair-transition bubble with independent work
                    for _ in range(2):
                        if feed:
                            feed.pop(0)()
                if pending_norm[0] is not None:
                    pending_norm[0]()
                    pending_norm[0] = None
                while feed:
                    feed.pop(0)()

            # ---- phase 1: projections for ALL chunks (PE-dense, no exp
            # chain to starve); xT DMAs for chunk n+1 are queued before
            # chunk n's matmuls so the transfers hide under compute
            xT_cur = xT_first
            QT_all = []
            for n in range(NCH):
                QT_n, s2g = make_s2_groups(n, xT_cur)
                if n + 1 < NCH:
                    xT_cur = emit_s1(n + 1)
                for g in s2g:
                    g()
                QT_all.append(QT_n)

            # ---- phase 2: attention, ACT(exp)-paced; feed only the light
            # F(n-1) groups into pair boundaries so scores are never starved
            f_prev = []
            for n in range(NCH):
                if '3' in stages:
                    emit_s3(n, QT_all[n], f_prev)
                else:
                    for g in f_prev:
                        g()
                    f_prev = []
                f_prev = make_f_groups(n)
            for g in f_prev:
                g()

        if repeat > 1:
            with tc.For_i(0, repeat, 1):
                emit_kernel()
        else:
            emit_kernel()

    nc.compile()
    return nc


_NC_CACHE = {}


def _get_nc(seq_len, causal):
    key = (seq_len, causal)
    if key not in _NC_CACHE:
        _NC_CACHE[key] = build_bass(seq_len, causal)
    return _NC_CACHE[key]


def make_in_maps(x, w_q, b_q, w_k, w_v, w_o):
    """Per-core input dicts for the 8 cores (weights/x pre-cast to bf16)."""
    import ml_dtypes
    bf = ml_dtypes.bfloat16
    xt_bf = np.ascontiguousarray(x.transpose(0, 2, 1)).astype(bf)
    wq_bf = w_q.astype(bf)
    wk_bf = w_k.astype(bf)
    wv_bf = w_v.astype(bf)
    wo_bf = w_o.astype(bf)
    in_maps = []
    for c in range(N_CORES):
        b, g = divmod(c, 2)
        sl = slice(g * DH, (g + 1) * DH)
        in_maps.append({
            "xt": xt_bf[b],
            "wq": np.ascontiguousarray(wq_bf[:, sl]),
            "wk": np.ascontiguousarray(wk_bf[:, sl]),
            "wv": np.ascontiguousarray(wv_bf[:, sl]),
            "wo": np.ascontiguousarray(wo_bf[sl, :]),
            "bq": np.ascontiguousarray(
                b_q[sl].reshape(DH // 128, 128).T.astype(np.float32)),
        })
    return in_maps


def kernel(x, mask, w_q, b_q, w_k, b_k, w_v, b_v, w_o, b_o, _trace=False):
    x = np.asarray(x, dtype=np.float32)
    mask_np = np.asarray(mask).reshape(mask.shape[-2], mask.shape[-1])
    w_q, b_q = np.asarray(w_q, np.float32), np.asarray(b_q, np.float32)
    w_k = np.asarray(w_k, np.float32)
    w_v, b_v = np.asarray(w_v, np.float32), np.asarray(b_v, np.float32)
    w_o, b_o = np.asarray(w_o, np.float32), np.asarray(b_o, np.float32)

    T = x.shape[1]
    tril = np.tril(np.ones((T, T), dtype=mask_np.dtype))
    if np.array_equal(mask_np, tril):
        causal = True
    elif np.all(mask_np != 0):
        causal = False
    else:
        raise NotImplementedError("only causal or all-ones masks supported")

    nc = _get_nc(T, causal)
    in_maps = make_in_maps(x, w_q, b_q, w_k, w_v, w_o)
    res = run_bass_kernel_spmd(nc, in_maps, core_ids=list(range(N_CORES)),
                               trace=_trace)

    host_bias = (b_v @ w_o + b_o).astype(np.float32)
    out = np.empty((x.shape[0], T, D_MODEL), dtype=np.float32)
    for b in range(x.shape[0]):
        out[b] = res.results[2 * b]["out"].astype(np.float32) \
            + res.results[2 * b + 1]["out"].astype(np.float32) + host_bias
    kernel._last_result = res
    return out



# revision 37
# speedup vs baseline: 1.1233x; 1.0145x over previous
"""Multi-head causal attention (B=4, T=2048, D=1024, H=16) on 8 trn2 NeuronCores.

Sharding: core c handles batch b = c//2 and head-group g = c%2 (8 heads each).
Each core computes Q/K/V projections for its 8 heads, causal attention, and a
row-shard of the output projection; the host sums the two partial outputs per
batch (the "all-reduce") and adds the (b_v @ w_o + b_o) bias term.

Device algebra notes:
  - b_k drops out of softmax entirely (adds a per-query constant to scores).
  - b_v contributes exactly (b_v @ w_o) to every output row -> folded into the
    host-side bias along with b_o.
  - Scores are computed transposed ([k, q] layout) so that softmax sums ride
    free on the AV matmul (ones-column appended to V) and the AV output comes
    out as AV^T, which feeds the w_o matmul with no extra transposes.

DMA rule: hardware DMA descriptors encode at most ONE semaphore wait, so every
DMA destination here is written exactly once (no pool-slot reuse for DMA
targets); partition broadcasts are done with PE outer products, not DMA.
"""

import math
from contextlib import ExitStack

import numpy as np

import concourse.bass as bass
import concourse.mybir as mybir
import concourse.tile as tile
from concourse import bacc
from concourse.bass_utils import run_bass_kernel_spmd


FP = mybir.dt.float32
BF = mybir.dt.bfloat16

D_MODEL = 1024
N_HEADS = 16
B_FULL, T_FULL = 4, 2048
DK = 64                    # head dim
HPC = 8                    # heads per core
DH = HPC * DK              # 512 head-dims per core
N_CORES = 8


def build_bass(seq_len=T_FULL, causal=True, repeat=1, stages='123F'):
    """Build the per-core Bass kernel (SPMD; same NEFF on all 8 cores).

    repeat > 1 wraps the whole kernel in a hardware loop — used only for
    benchmarking (amortizes host dispatch to time the kernel itself).
    """
    T = seq_len
    NT = T // 128             # t-tiles
    NCH = T // 512            # 512-wide t/q chunks
    ND = D_MODEL // 128       # d_model tiles (8)
    NM = DH // 128            # head-pair tiles (4)

    nc = bacc.Bacc("TRN2", target_bir_lowering=False, debug=False)
    # x arrives pre-transposed AND pre-cast to bf16 (host does both), so the
    # xT tile loads are plain partition-contiguous DMAs, not transpose DMAs.
    xt_d = nc.dram_tensor("xt", [D_MODEL, T], BF, kind="ExternalInput")
    wq_d = nc.dram_tensor("wq", [D_MODEL, DH], BF, kind="ExternalInput")
    wk_d = nc.dram_tensor("wk", [D_MODEL, DH], BF, kind="ExternalInput")
    wv_d = nc.dram_tensor("wv", [D_MODEL, DH], BF, kind="ExternalInput")
    wo_d = nc.dram_tensor("wo", [DH, D_MODEL], BF, kind="ExternalInput")
    bq_d = nc.dram_tensor("bq", [128, DH // 128], FP, kind="ExternalInput")
    out_d = nc.dram_tensor("out", [T, D_MODEL], BF, kind="ExternalOutput")

    with ExitStack() as ctx:
        tc = ctx.enter_context(tile.TileContext(nc))
        persist = ctx.enter_context(tc.tile_pool(name="persist", bufs=1))
        qt_pool = ctx.enter_context(tc.tile_pool(name="qt", bufs=3))
        xt_pool = ctx.enter_context(tc.tile_pool(name="xt", bufs=3))
        at_pool = ctx.enter_context(tc.tile_pool(name="atp", bufs=6))
        rec_pool = ctx.enter_context(tc.tile_pool(name="rec", bufs=2))
        avn_pool = ctx.enter_context(tc.tile_pool(name="avn", bufs=2))
        out_pool = ctx.enter_context(tc.tile_pool(name="outp", bufs=3))
        mm_ps = ctx.enter_context(tc.tile_pool(name="mmps", bufs=2, space="PSUM"))
        sc_ps = ctx.enter_context(tc.tile_pool(name="scps", bufs=2, space="PSUM"))
        av_ps = ctx.enter_context(tc.tile_pool(name="avps", bufs=2, space="PSUM"))

        def emit_kernel():
            # ---- constants -------------------------------------------------
            ones_bf = persist.tile([1, DK], BF, name="ones_bf", tag="ones_bf")
            nc.gpsimd.memset(ones_bf, 1.0)

            mask_tri = None
            if causal:
                # one shared lower-triangular keep mask for the diagonal
                # 128x128 block: keep where q_local >= k
                mask_tri = persist.tile([128, 128], BF, name="mask_tri",
                                        tag="mask_tri")
                nc.gpsimd.memset(mask_tri, 1.0)
                nc.gpsimd.affine_select(
                    out=mask_tri, in_=mask_tri,
                    compare_op=mybir.AluOpType.is_ge,
                    fill=0.0, base=0, pattern=[[1, 128]],
                    channel_multiplier=-1)

            def bcast_mid(ap, nmid):
                return bass.AP(tensor=ap.tensor, offset=ap.offset,
                               ap=[list(ap.ap[0]), [0, nmid], list(ap.ap[1])])

            def emit_s1(n):
                xT = [xt_pool.tile([128, 512], BF, name=f"xT{j}",
                                   tag=f"xT{j}") for j in range(ND)]
                for j in range(ND):
                    nc.sync.dma_start(out=xT[j],
                                      in_=xt_d[j * 128:(j + 1) * 128,
                                               n * 512:(n + 1) * 512])
                return xT

            # queue chunk-0 x tiles ahead of the weights: the first kgroup
            # needs xT(0) and wk, and the 8 HWDGE queues drain round-robin
            xT_first = emit_s1(0)

            # ---- weights: one DMA per 128-row tile, spread across the 8
            # HWDGE queues (per-queue bandwidth is ~1/16 of aggregate, so a
            # single 1MB DMA would serialize ~26us at startup)
            def load_weight_bf(dram, n_tiles, cols, label):
                wsb = persist.tile([128, n_tiles, cols], BF, name=f"w_{label}",
                                   tag=f"w_{label}")
                dap = dram.ap().rearrange("(j p) c -> p j c", p=128)
                for j in range(n_tiles):
                    nc.sync.dma_start(out=wsb[:, j, :], in_=dap[:, j, :])
                return [wsb[:, j, :] for j in range(n_tiles)]

            wk_bf = load_weight_bf(wk_d, ND, DH, "k")
            wq_bf = load_weight_bf(wq_d, ND, DH, "q")
            wv_bf = load_weight_bf(wv_d, ND, DH, "v")
            wo_bf = load_weight_bf(wo_d, NM, D_MODEL, "o")

            bq_sb = persist.tile([128, NM], FP, name="bq_sb", tag="bq_sb")
            nc.sync.dma_start(out=bq_sb, in_=bq_d[:, :])

            # ---- persistent per-core tensors ------------------------------
            # KT[mt]: [128, T] bf16, rows = head-dim (pair mt: heads 2mt,2mt+1)
            KT = [persist.tile([128, T], BF, name=f"KT{mt}", tag=f"KT{mt}")
                  for mt in range(NM)]
            # V natural layout with ones column: [128 t, 8 heads, 64+1]
            V_sb = [persist.tile([128, HPC, DK + 1], BF, name=f"V{i}", tag=f"V{i}")
                    for i in range(NT)]
            for i in range(NT):
                nc.gpsimd.memset(V_sb[i][:, :, DK], 1.0)
            # AV^T, write-once (DMA target for the h1 partition shift)
            AVT_all = [[persist.tile([128, 512], BF, name=f"AVT{n}_{p}",
                                     tag=f"AVT{n}_{p}") for p in range(NM)]
                       for n in range(NCH)]

            # ---- main streamed loop over 512-wide chunks -------------------
            # S2(n+1) and F(n-1) matmul groups are fed into S3(n)'s pair
            # boundaries: the per-engine queues are in-order, so S3's
            # exp-latency bubbles can only be filled by positionally
            # interleaving independent work into its instruction stream.

            def make_s2_groups(n, xT):
                csl = bass.ts(n, 512)
                QT = [persist.tile([128, 512], BF, name=f"QT{n}_{mt}",
                                   tag=f"QT{n}_{mt}") for mt in range(NM)]
                groups = []

                def qgroup(mt):
                    msl = bass.ts(mt, 128)
                    ps = mm_ps.tile([128, 512], FP, name="psq", tag="mm")
                    for j in range(ND):
                        nc.tensor.matmul(ps, wq_bf[j][:, msl], xT[j],
                                         start=(j == 0), stop=(j == ND - 1))
                    nc.vector.tensor_scalar_add(QT[mt], ps, bq_sb[:, mt:mt + 1])

                def kgroup(mt):
                    msl = bass.ts(mt, 128)
                    ps = mm_ps.tile([128, 512], FP, name="psk", tag="mm")
                    for j in range(ND):
                        nc.tensor.matmul(ps, wk_bf[j][:, msl], xT[j],
                                         start=(j == 0), stop=(j == ND - 1))
                    # fold the 1/sqrt(dk) score scale into K^T so exp needs
                    # no scale and score magnitudes stay ~N(0,1)
                    nc.vector.tensor_scalar_mul(KT[mt][:, csl], ps,
                                                1.0 / math.sqrt(DK))

                def vgroup(il):
                    i = 4 * n + il
                    ps = mm_ps.tile([128, 512], FP, name="psv", tag="mm")
                    for j in range(ND):
                        nc.tensor.matmul(ps, xT[j][:, il * 128:(il + 1) * 128],
                                         wv_bf[j], start=(j == 0),
                                         stop=(j == ND - 1))
                    nc.vector.tensor_copy(
                        V_sb[i][:, :, 0:DK],
                        ps.rearrange("p (h d) -> p h d", h=HPC))

                if '2' in stages:
                    for mt in range(NM):
                        groups.append(lambda mt=mt: kgroup(mt))
                        groups.append(lambda mt=mt: qgroup(mt))
                    for il in range(4):
                        groups.append(lambda il=il: vgroup(il))
                return QT, groups

            def make_f_groups(n):
                AVT = AVT_all[n]
                groups = []

                osbs = {}

                def fgroup(il, cc):
                    i = 4 * n + il
                    isl = bass.ts(il, 128)
                    if il not in osbs:
                        osbs[il] = out_pool.tile([128, D_MODEL], BF,
                                                 name="osb", tag="osb")
                    osb = osbs[il]
                    ps = mm_ps.tile([128, 512], FP, name="pso", tag="mm")
                    for dk in range(NM):
                        nc.tensor.matmul(
                            ps, AVT[dk][:, isl],
                            wo_bf[dk][:, cc * 512:(cc + 1) * 512],
                            start=(dk == 0), stop=(dk == NM - 1))
                    csl2 = slice(cc * 512, (cc + 1) * 512)
                    nc.vector.tensor_copy(osb[:, csl2], ps)
                    # half-width bf16 DMA per cc: keeps per-queue occupancy
                    # short so xT prefetches behind it aren't head-blocked
                    nc.sync.dma_start(
                        out=out_d[i * 128:(i + 1) * 128, csl2],
                        in_=osb[:, csl2])

                if 'F' in stages:
                    for il in range(4):
                        for cc in range(2):
                            groups.append(lambda il=il, cc=cc: fgroup(il, cc))
                return groups

            def emit_s3(n, QT, feed):
                AVT = AVT_all[n]
                nkt = 4 * n + 4 if causal else NT
                PIPE = 3
                pending_norm = [None]

                def emit_norm(p, av0, av1):
                    den_bf = rec_pool.tile([1, 2, 512], BF, name="den_bf",
                                           tag="den_bf")
                    nc.vector.tensor_copy(den_bf[:, 0, :], av0[DK:DK + 1, :])
                    nc.vector.tensor_copy(den_bf[:, 1, :], av1[DK:DK + 1, :])
                    rb_sb = rec_pool.tile([DK, 2, 512], FP, name="rb_sb",
                                          tag="rb_sb")
                    for hh in range(2):
                        rb = mm_ps.tile([DK, 512], FP, name=f"rb{hh}",
                                        tag="mm")
                        nc.tensor.matmul(rb, ones_bf, den_bf[:, hh, :],
                                         start=True, stop=True)
                        nc.vector.reciprocal(rb_sb[:, hh, :], rb)
                    nc.vector.tensor_mul(AVT[p][0:64, :], av0[0:DK, :],
                                         rb_sb[:, 0, :])
                    avn1 = avn_pool.tile([64, 512], BF, name="avn1",
                                         tag="avn1")
                    nc.vector.tensor_mul(avn1, av1[0:DK, :], rb_sb[:, 1, :])
                    nc.sync.dma_start(out=AVT[p][64:128, :], in_=avn1)

                npairs = NM if '3' in stages else 0
                for p in range(npairs):
                    av0 = av_ps.tile([DK + 1, 512], FP, name="av0", tag="av")
                    av1 = av_ps.tile([DK + 1, 512], FP, name="av1", tag="av")
                    avs = (av0, av1)

                    def emit_av(idx, kt, at, q0):
                        for hh in range(2):
                            nc.tensor.matmul(
                                avs[hh][:, q0:512],
                                V_sb[kt][:, 2 * p + hh, :],
                                at[:, hh, q0:512],
                                start=(idx == 0), stop=(idx == nkt - 1),
                                skip_group_check=True)

                    # process diagonal (masked, partial-width) tiles FIRST so
                    # their longer exp+mask chains hide under the full tiles;
                    # diag j=0 is full-width so start= covers all columns
                    if causal:
                        kt_order = list(range(4 * n, nkt)) + list(range(4 * n))
                    else:
                        kt_order = list(range(nkt))

                    pend = []
                    for idx, kt in enumerate(kt_order):
                        ksl = bass.ts(kt, 128)
                        # pop the ready AV FIRST: the PE queue is in-order,
                        # so a scores matmul blocked on its psum bank must
                        # not sit ahead of AV work that can already run
                        if len(pend) > PIPE - 1:
                            emit_av(*pend.pop(0))
                        # columns < j*128 of a diagonal tile are fully
                        # masked: skip in scores matmul / exp / mask / AV
                        j = kt - 4 * n if (causal and kt >= 4 * n) else 0
                        q0 = j * 128
                        ps_s = sc_ps.tile([128, 2, 512], FP, name="ps_s",
                                          tag="sc")
                        at = at_pool.tile([128, 2, 512], BF, name="at",
                                          tag="at")
                        for hh in range(2):
                            nc.tensor.matmul(
                                ps_s[:, hh, q0:512],
                                KT[p][hh * 64:(hh + 1) * 64, ksl],
                                QT[p][hh * 64:(hh + 1) * 64, q0:512],
                                start=True, stop=True,
                                tile_position=(hh * 64, 0))
                        nc.scalar.activation(at[:, :, q0:512],
                                             ps_s[:, :, q0:512],
                                             mybir.ActivationFunctionType.Exp)
                        if causal and kt >= 4 * n:
                            # only the diagonal 128-col block needs masking;
                            # columns right of it are fully unmasked
                            nc.vector.tensor_mul(
                                at[:, :, q0:q0 + 128], at[:, :, q0:q0 + 128],
                                bcast_mid(mask_tri[:, :], 2))
                        # deferred norm of the previous pair: fire a couple of
                        # steps in so its PE broadcasts (waiting on DVE den
                        # copies) don't head-block this pair's early scores
                        if idx == min(2, nkt - 1) and \
                                pending_norm[0] is not None:
                            pending_norm[0]()
                            pending_norm[0] = None
                        pend.append((idx, kt, at, q0))
                    for item in pend:
                        emit_av(*item)
                    pending_norm[0] = (lambda p=p, a0=av0, a1=av1:
                                       emit_norm(p, a0, a1))
                    # fill the pair-transition bubble with independent work
                    for _ in range(5):
                        if feed:
                            feed.pop(0)()
                if pending_norm[0] is not None:
                    pending_norm[0]()
                    pending_norm[0] = None
                while feed:
                    feed.pop(0)()

            xT_cur = xT_first
            QT_cur, s2g = make_s2_groups(0, xT_cur)
            for g in s2g:
                g()
            f_prev = []
            for n in range(NCH):
                feed = []
                if n + 1 < NCH:
                    xT_next = emit_s1(n + 1)
                    QT_next, s2g_next = make_s2_groups(n + 1, xT_next)
                    feed += s2g_next
                else:
                    QT_next = None
                feed += f_prev
                if '3' in stages:
                    emit_s3(n, QT_cur, feed)
                else:
                    for g in feed:
                        g()
                f_prev = make_f_groups(n)
                QT_cur = QT_next
            for g in f_prev:
                g()

        if repeat > 1:
            with tc.For_i(0, repeat, 1):
                emit_kernel()
        else:
            emit_kernel()

    nc.compile()
    return nc


_NC_CACHE = {}


def _get_nc(seq_len, causal):
    key = (seq_len, causal)
    if key not in _NC_CACHE:
        _NC_CACHE[key] = build_bass(seq_len, causal)
    return _NC_CACHE[key]


def make_in_maps(x, w_q, b_q, w_k, w_v, w_o):
    """Per-core input dicts for the 8 cores (weights/x pre-cast to bf16)."""
    import ml_dtypes
    bf = ml_dtypes.bfloat16
    xt_bf = np.ascontiguousarray(x.transpose(0, 2, 1)).astype(bf)
    wq_bf = w_q.astype(bf)
    wk_bf = w_k.astype(bf)
    wv_bf = w_v.astype(bf)
    wo_bf = w_o.astype(bf)
    in_maps = []
    for c in range(N_CORES):
        b, g = divmod(c, 2)
        sl = slice(g * DH, (g + 1) * DH)
        in_maps.append({
            "xt": xt_bf[b],
            "wq": np.ascontiguousarray(wq_bf[:, sl]),
            "wk": np.ascontiguousarray(wk_bf[:, sl]),
            "wv": np.ascontiguousarray(wv_bf[:, sl]),
            "wo": np.ascontiguousarray(wo_bf[sl, :]),
            "bq": np.ascontiguousarray(
                b_q[sl].reshape(DH // 128, 128).T.astype(np.float32)),
        })
    return in_maps


def kernel(x, mask, w_q, b_q, w_k, b_k, w_v, b_v, w_o, b_o, _trace=False):
    x = np.asarray(x, dtype=np.float32)
    mask_np = np.asarray(mask).reshape(mask.shape[-2], mask.shape[-1])
    w_q, b_q = np.asarray(w_q, np.float32), np.asarray(b_q, np.float32)
    w_k = np.asarray(w_k, np.float32)
    w_v, b_v = np.asarray(w_v, np.float32), np.asarray(b_v, np.float32)
    w_o, b_o = np.asarray(w_o, np.float32), np.asarray(b_o, np.float32)

    T = x.shape[1]
    tril = np.tril(np.ones((T, T), dtype=mask_np.dtype))
    if np.array_equal(mask_np, tril):
        causal = True
    elif np.all(mask_np != 0):
        causal = False
    else:
        raise NotImplementedError("only causal or all-ones masks supported")

    nc = _get_nc(T, causal)
    in_maps = make_in_maps(x, w_q, b_q, w_k, w_v, w_o)
    res = run_bass_kernel_spmd(nc, in_maps, core_ids=list(range(N_CORES)),
                               trace=_trace)

    host_bias = (b_v @ w_o + b_o).astype(np.float32)
    out = np.empty((x.shape[0], T, D_MODEL), dtype=np.float32)
    for b in range(x.shape[0]):
        out[b] = res.results[2 * b]["out"].astype(np.float32) \
            + res.results[2 * b + 1]["out"].astype(np.float32) + host_bias
    kernel._last_result = res
    return out

